# revision 1
# baseline (speedup 1.0000x reference)
"""GCN layer (normalized-adjacency aggregation) on 8 Trainium2 NeuronCores.

kernel(x, adj, weight, bias) -> out, computing (fp32 I/O):
    deg = adj.sum(axis=1); d = deg**-0.5 (0 where deg == 0)
    out = diag(d) @ adj @ diag(d) @ (x @ weight) + bias

Distribution: adj is row-sharded across the 8 cores ([1024, 8192] block per
core); x^T / weight / bias are replicated. Per core the kernel streams its
adj block ONCE:
  - SWDGE cast-DMA fp32->fp16 per 128-row strip; DVE row-sums each strip
    for the local deg (fp32 accumulate); an xbar DMA-transpose writes each
    strip into an SBUF-resident fp16 stash of the transposed block,
    strip-major (stash[p, rs, c, il] = adj[rs*128+il, c*128+p]) so every
    transpose writes one contiguous per-partition run (fast xbar path).
  - The deg exchange is split into 5 partial AllGathers over strip groups
    [2, 2, 2, 1, 1] so d (= rsqrt(deg), Newton-refined, deg==0 guarded) for
    early chunks is ready while later strips still stream, and the final
    serial round covers only one strip (streamed in half-width pieces).
  - Support S = x @ weight runs under the stream (fp16 matmuls, lhsT =
    host-transposed fp16 x chunks), stored fp16, scaled in-place by d.
  - Aggregation psum[ib] += stash[:, ib, c, :].T @ S'[:, c, :] is
    emitted in dependency-ready order (engines execute in program order),
    so most of it overlaps the stream; the epilogue applies the local row
    scale and bias in one DVE op.
Host slices inputs, runs the SPMD program on cores 0-7, concatenates the
8 output shards.
"""

import sys

sys.path.insert(0, "/opt/trn_rl_repo")

import numpy as np

import concourse.bacc as bacc
import concourse.mybir as mybir
import concourse.tile as tile
from concourse import bass_utils

F32 = mybir.dt.float32
F16 = mybir.dt.float16
AX = mybir.AxisListType
OP = mybir.AluOpType

N, M, F = 8192, 8, 256
R = N // M          # 1024 local rows per core
NS = R // 128       # 8 row strips per core
C = N // 128        # 64 contraction chunks
CCK = F // 128      # 2 x-feature chunks
G_CS = 16           # S-matmul chunks per xT piece
NG = C // G_CS      # 4 xT pieces
ROUNDS = [2, 2, 2, 1, 1]
NR = len(ROUNDS)
RSTART = [sum(ROUNDS[:g]) for g in range(NR)]
ROUND_OF_STRIP = []
for _g, _n in enumerate(ROUNDS):
    ROUND_OF_STRIP += [_g] * _n


def _rsqrt(nc, out_ap, in_ap, s0, s1):
    """out = in**-0.5 with (in==0 -> 0), Newton-refined. May alias in/out."""
    nc.vector.tensor_scalar(s1, in_ap, 1e30, 1.0, op0=OP.mult, op1=OP.min)
    nc.vector.tensor_scalar_max(s0, in_ap, 1e-30)
    nc.scalar.sqrt(s0, s0)
    nc.vector.reciprocal(s0, s0)
    nc.vector.tensor_mul(out_ap, in_ap, s0)
    nc.vector.tensor_mul(out_ap, out_ap, s0)
    nc.vector.tensor_scalar(out_ap, out_ap, -0.5, 1.5, op0=OP.mult, op1=OP.add)
    nc.vector.tensor_mul(out_ap, out_ap, s0)
    nc.vector.tensor_mul(out_ap, out_ap, s1)


def _build_nc():
    nc = bacc.Bacc("TRN2", target_bir_lowering=False, debug=False, num_devices=M)

    adj = nc.dram_tensor("adj", [R, N], F32, kind="ExternalInput").ap()
    xT = nc.dram_tensor("xT", [F, N], F16, kind="ExternalInput").ap()
    w = nc.dram_tensor("w", [F, F], F16, kind="ExternalInput").ap()
    bias_rep = nc.dram_tensor("bias_rep", [128, F], F32, kind="ExternalInput").ap()
    out = nc.dram_tensor("out", [R, F], F32, kind="ExternalOutput").ap()

    with tile.TileContext(nc) as tc:
        with (
            tc.tile_pool(name="dram", bufs=1, space="DRAM") as dram,
            tc.tile_pool(name="persist", bufs=1) as persist,
            tc.tile_pool(name="stream", bufs=2) as stream,
            tc.tile_pool(name="xtp", bufs=1) as xtp,
            tc.tile_pool(name="outp", bufs=2) as outp,
        ):
            stash = persist.tile([128, NS, C, 128], F16)  # adj^T, strip-major
            sprime = persist.tile([128, C, F], F16)    # S then S' rows
            w_sb = persist.tile([128, CCK, F], F16)
            bias_sb = persist.tile([128, F], F32)
            deg_sb = persist.tile([128, NS], F32)
            degl = persist.tile([128, 2, 2], F32)
            d_sb = persist.tile([128, C], F32)
            d_loc = persist.tile([128, NS], F32)
            scr0 = persist.tile([128, C], F32)
            scr1 = persist.tile([128, C], F32)

            nc.scalar.dma_start(w_sb[:], w.rearrange("(cc p) f -> p cc f", p=128))
            nc.scalar.dma_start(bias_sb[:], bias_rep[:])

            psum_s = tc.alloc_tile_pool(name="psum_s", bufs=2, space="PSUM")
            psum_state = {}

            def get_psum_outs():
                # allocated lazily, after psum_s is released (stack order)
                if "pa" not in psum_state:
                    pool = tc.alloc_tile_pool(name="psum_a", bufs=NS, space="PSUM")
                    psum_state["pool"] = pool
                    psum_state["pa"] = [
                        pool.tile([128, F], F32, tag="pa", name=f"pa{b}")
                        for b in range(NS)
                    ]
                return psum_state["pa"]

            def emit_support(pieces):
                for g in pieces:
                    xt_sb = xtp.tile(
                        [128, CCK, G_CS * 128], F16, tag="xt", name=f"xt{g}"
                    )
                    nc.scalar.dma_start(
                        xt_sb[:],
                        xT.rearrange("(cc p) i -> p cc i", p=128)[
                            :, :, g * G_CS * 128 : (g + 1) * G_CS * 128
                        ],
                    )
                    for cl in range(G_CS):
                        c = g * G_CS + cl
                        ps = psum_s.tile([128, F], F32, tag="ps", name=f"ps{c}")
                        for cc in range(CCK):
                            nc.tensor.matmul(
                                ps[:],
                                xt_sb[:, cc, cl * 128 : (cl + 1) * 128],
                                w_sb[:, cc, :],
                                start=(cc == 0),
                                stop=(cc == CCK - 1),
                            )
                        nc.scalar.copy(sprime[:, c, :], ps[:])

            mm_started = [False] * NS
            n_mm_left = [C] * NS

            def emit_stream_round(g):
                for rs in range(RSTART[g], RSTART[g] + ROUNDS[g]):
                    # the final strips gate the serial tail: stream them in
                    # two half-width pieces so transposes/deg land earlier
                    npieces = 2 if rs >= NS - 2 else 1
                    pcols = N // npieces
                    pchunks = pcols // 128
                    for pp in range(npieces):
                        nat = stream.tile(
                            [128, pcols], F16, tag="nat", padded_shape=[128, N]
                        )
                        nc.gpsimd.dma_start(
                            nat[:],
                            adj[
                                rs * 128 : (rs + 1) * 128,
                                pp * pcols : (pp + 1) * pcols,
                            ],
                        )
                        red_out = (
                            deg_sb[:, rs : rs + 1]
                            if npieces == 1
                            else degl[:, rs % 2, pp : pp + 1]
                        )
                        nc.vector.tensor_reduce(
                            red_out, nat[:], axis=AX.X, op=OP.add
                        )
                        nc.sync.dma_start(
                            stash[:, rs, pp * pchunks : (pp + 1) * pchunks, :],
                            nat[:],
                            transpose=True,
                        )
                    if npieces > 1:
                        nc.vector.tensor_reduce(
                            deg_sb[:, rs : rs + 1],
                            degl[:, rs % 2, :npieces],
                            axis=AX.X,
                            op=OP.add,
                        )
                spg = ROUNDS[g]
                deg_local = dram.tile(
                    [spg * 128], F32, tag=f"dl{g}", name=f"deg_local{g}"
                )
                deg_full = dram.tile(
                    [M * spg * 128],
                    F32,
                    addr_space="Shared",
                    tag=f"df{g}",
                    name=f"deg_full{g}",
                )
                nc.scalar.dma_start(
                    deg_local.rearrange("(s q) -> q s", q=128),
                    deg_sb[:, RSTART[g] : RSTART[g] + spg],
                )
                nc.gpsimd.collective_compute(
                    "AllGather",
                    OP.bypass,
                    replica_groups=[list(range(M))],
                    ins=[deg_local.opt()],
                    outs=[deg_full.opt()],
                )
                return deg_full

            def emit_d_round(g, deg_full):
                spg = ROUNDS[g]
                dfv = deg_full.rearrange("(m s q) -> q m s", q=128, s=spg)
                for k in range(spg):
                    cs = slice(RSTART[g] + k, C, NS)
                    nc.scalar.dma_start(d_sb[:, cs], dfv[:, :, k])
                    _rsqrt(nc, d_sb[:, cs], d_sb[:, cs], scr0[:, cs], scr1[:, cs])
                for m in range(M):
                    for k in range(spg):
                        c = m * NS + RSTART[g] + k
                        nc.scalar.mul(
                            sprime[:, c, :], sprime[:, c, :], d_sb[:, c : c + 1]
                        )

            def emit_mm(gc, ib):
                for m in range(M):
                    for k in range(ROUNDS[gc]):
                        c = m * NS + RSTART[gc] + k
                        n_mm_left[ib] -= 1
                        nc.tensor.matmul(
                            get_psum_outs()[ib][:],
                            stash[:, ib, c, :],
                            sprime[:, c, :],
                            start=(not mm_started[ib]),
                            stop=(n_mm_left[ib] == 0),
                        )
                        mm_started[ib] = True

            def emit_step(s):
                # pairs (gc, ib) with max(gc+1, round(ib)) == s; MMs gated on
                # the newest strips come last (engines run in program order)
                if s - 1 < NR:
                    for ib in [i for i in range(NS) if ROUND_OF_STRIP[i] <= s - 1]:
                        emit_mm(s - 1, ib)
                if s < NR:
                    for gc in range(min(s, NR)):
                        for ib in [i for i in range(NS) if ROUND_OF_STRIP[i] == s]:
                            emit_mm(gc, ib)

            emit_support(range(NG))
            psum_s.release()

            gathers = []
            for g in range(NR):
                gathers.append(emit_stream_round(g))
                if g >= 1:
                    emit_d_round(g - 1, gathers[g - 1])
                    emit_step(g)
            emit_d_round(NR - 1, gathers[NR - 1])
            emit_step(NR)

            _rsqrt(nc, d_loc[:], deg_sb[:], scr0[:, :NS], scr1[:, :NS])
            for ib in range(NS):
                ot = outp.tile([128, F], F32, tag="ot")
                nc.vector.scalar_tensor_tensor(
                    ot[:],
                    get_psum_outs()[ib][:],
                    d_loc[:, ib : ib + 1],
                    bias_sb[:],
                    op0=OP.mult,
                    op1=OP.add,
                )
                nc.sync.dma_start(out[ib * 128 : (ib + 1) * 128, :], ot[:])

            assert all(v == 0 for v in n_mm_left), n_mm_left
            psum_state["pool"].release()

    nc.compile()
    return nc


_NC_CACHE = None


def _get_nc():
    global _NC_CACHE
    if _NC_CACHE is None:
        _NC_CACHE = _build_nc()
    return _NC_CACHE


def kernel(x, adj, weight, bias):
    x = np.asarray(x, dtype=np.float32)
    adj = np.asarray(adj, dtype=np.float32)
    weight = np.asarray(weight, dtype=np.float32)
    bias = np.asarray(bias, dtype=np.float32)

    nc = _get_nc()

    xTh = x.T.astype(np.float16)  # astype returns C-contiguous
    wh = weight.astype(np.float16)
    bh = np.ascontiguousarray(np.broadcast_to(bias, (128, F)))
    in_maps = [
        {
            "adj": np.ascontiguousarray(adj[m * R : (m + 1) * R, :]),
            "xT": xTh,
            "w": wh,
            "bias_rep": bh,
        }
        for m in range(M)
    ]

    # retry once on transient device faults (e.g. NRT_EXEC_UNIT_UNRECOVERABLE
    # seen sporadically on the shared axon terminal)
    last_exc = None
    for _attempt in range(3):
        try:
            res = bass_utils.run_bass_kernel_spmd(
                nc, in_maps, core_ids=list(range(M))
            )
            return np.concatenate(
                [res.results[m]["out"] for m in range(M)], axis=0
            )
        except Exception as e:  # noqa: BLE001
            last_exc = e
    raise last_exc

